# revision 35
# baseline (speedup 1.0000x reference)
"""LocalAttention3D Trainium2 kernel (Gram-factored).

Problem: x [B=2, C=1, D=96, H=64, W=64], per-head scalar-affine q/k/v
projections (NH=4 heads), scores = einsum('bdjk,bdlm->bjklm', q, k)/sqrt(32),
softmax over the last W axis only (windows of 64), out = attn @ v, then sum
over heads.

Key algebraic restructuring: with q = wq*x+bq, k = wk*x+bk the score is
  s_h[jk, lm] = wq_h*wk_h * G[jk, lm] + wq_h*bk_h * r[jk]
              + bq_h*wk_h * r[lm] + 96*bq_h*bk_h,
where G = X^T X (head-independent Gram matrix) and r = X.sum(depth). The
softmax normalizes over m (within lm), so every term constant in m (the
r[jk] term and the constant) cancels. The r[lm] term factors out of the
softmax as f_h[lm] = exp(SCALE*bq_h*wk_h*r[lm]) and is folded
multiplicatively into the ones-matrix (denominator sums) and into V
(numerator), exactly preserving the attention weights:
  out = sum_lm V[d,lm]*f[lm]*E0[lm,jk] / (window-sum_m f*E0),
  E0 = exp(SCALE*alpha_h*G).
So the only large matmul producing scores is the Gram matrix — computed
ONCE, shared by all 4 heads — and it runs in float32r (full PE rate for
moving dim >= 256 vs 4x slowdown for plain f32).

Sharding: core = (batch b, jk-quarter). Each core computes the full lm
range of G columns for its 1024 jk columns (G shards cleanly by columns),
runs all 4 heads, and accumulates the head sum directly in the AV PSUM
tile. Output blocks [96, 1024] are disjoint: no reduction anywhere.

Per-core pipeline over 16 "super-tiles" (2 lm-tiles of 128 partitions
each; G super-tile [128, 2048] = both tiles' 1024 jk columns side by
side in the free dim):
  it st:  PE: Gram(st) [4 x 512-col f32r matmuls into PSUM]
          ACT: E0(st,h) = exp(scl_h * G(st)) -> bf16 SBUF   (4 heads)
          PE: Z(st-1): per (h, half): bo^T @ E0 -> 2-row stripes of
              zf PSUM (f-weighted window sums; bo carries f)
          DVE (group boundary): reciprocal of zf -> zi bf16
          DMA: broadcast zi rows across 64 partitions -> zb
          DVE: pt = E0 * zb (bf16)
          PE: AV(st-3): vt_ht^T @ pt accumulated into av [96, 1024]
              across ALL (h, t) — the head sum is free.
Engine balance per core: PE ~295k cycles (~123 us), ACT 64 exps of
free-2048 (~122 us), DVE/DMA well under.
"""

import math
import sys

sys.path.insert(0, "/opt/trn_rl_repo")

import numpy as np
import ml_dtypes

import bass_rust
import concourse.bass as bass
import concourse.tile as tile
from concourse import mybir
from concourse.bass_utils import run_bass_kernel_spmd

BF16 = ml_dtypes.bfloat16

B, D, HW = 2, 96, 64 * 64
NH = 4
NCORES = 8
JQ = 1024             # jk columns per core (quarter of 4096)
NT = HW // 128        # 32 lm-tiles of 128 partitions
NST = NT // 2         # 16 super-tiles (2 lm-tiles each)
SCALE = 1.0 / math.sqrt(32.0)


def _split_excess_waits(nc, max_waits=1):
    """This container's walrus rejects instructions with >1 semaphore wait
    ("Too many sync wait commands"). Move extra waits onto no-op carriers
    inserted just before the instruction on the same engine."""
    ctr = 0
    for f in nc.m.functions:
        for blk in f.blocks:
            insts = blk.instructions
            out = []
            changed = False
            for ins in insts:
                try:
                    si = ins.sync_info
                except Exception:
                    si = None
                if si is not None and len(si.on_wait) > max_waits:
                    waits = list(si.on_wait)
                    for w in waits[:-max_waits]:
                        ctr += 1
                        nop = mybir.InstNoOp(
                            name=f"wsplit-{ctr}-{ins.name}", ins=[], outs=[])
                        nop.engine = ins.engine
                        nop.sync_info = bass_rust.SyncInfo(
                            on_wait=[w], on_update=[])
                        nc.register_instruction(nop, overwrite=True)
                        out.append(nop)
                        changed = True
                    ins.sync_info = bass_rust.SyncInfo(
                        on_wait=waits[-max_waits:], on_update=list(si.on_update))
                out.append(ins)
            if changed:
                blk.instructions = out


def _build_program():
    f32 = mybir.dt.float32
    f32r = mybir.dt.float32r
    bf16 = mybir.dt.bfloat16

    nc = bass.Bass("TRN2", target_bir_lowering=False, debug=False,
                   num_devices=1)
    xr_d = nc.dram_tensor("xr", [D, HW], f32r, kind="ExternalInput").ap()
    xq_d = nc.dram_tensor("xq", [D, JQ], f32r, kind="ExternalInput").ap()
    vt_d = nc.dram_tensor("vt", [128, NH * NT * D], bf16,
                          kind="ExternalInput").ap()
    bo_d = nc.dram_tensor("bo", [128, NH * NT * 32], bf16,
                          kind="ExternalInput").ap()
    scl_d = nc.dram_tensor("scl", [128, NH], f32, kind="ExternalInput").ap()
    out_d = nc.dram_tensor("out", [D, JQ], f32, kind="ExternalOutput").ap()

    with tile.TileContext(nc) as tc:
        with (
            tc.tile_pool(name="cn", bufs=1) as cn,
            tc.tile_pool(name="ew", bufs=17) as ew,
            tc.tile_pool(name="zn", bufs=2) as zn,
            tc.tile_pool(name="zb", bufs=9) as zbp,
            tc.tile_pool(name="pt", bufs=5) as ptp,
            tc.tile_pool(name="ob", bufs=1) as obp,
            tc.tile_pool(name="ps_g", bufs=1, space="PSUM") as ps_g,
            tc.tile_pool(name="ps_z", bufs=1, space="PSUM") as ps_z,
            tc.tile_pool(name="ps_av", bufs=1, space="PSUM") as ps_av,
        ):
            XR = cn.tile([D, HW], f32r, tag="XR")
            XQ = cn.tile([D, JQ], f32r, tag="XQ")
            VT = cn.tile([128, NH * NT * D], bf16, tag="VT")
            BO = cn.tile([128, NH * NT * 32], bf16, tag="BO")
            SCL = cn.tile([128, NH], f32, tag="SCL")
            nc.sync.dma_start(XQ[:], xq_d[:])
            for c4 in range(4):
                nc.sync.dma_start(XR[:, c4 * 1024:(c4 + 1) * 1024],
                                  xr_d[:, c4 * 1024:(c4 + 1) * 1024])
            nc.gpsimd.dma_start(SCL[:], scl_d[:])
            nc.gpsimd.dma_start(BO[:], bo_d[:])
            for c4 in range(4):
                nc.gpsimd.dma_start(
                    VT[:, c4 * NT * D:(c4 + 1) * NT * D],
                    vt_d[:, c4 * NT * D:(c4 + 1) * NT * D])

            av = ps_av.tile([D, JQ], f32, tag="av")

            e_tiles = {}     # st -> [4 E0 tiles]
            zf_tiles = {}    # group -> (zfA, zfB)
            zi_tiles = {}    # group -> zi

            def emit_gram_exp(st):
                G = ps_g.tile([128, 2048], f32, tag="G")
                for half in (0, 1):
                    t = 2 * st + half
                    for q in (0, 1):
                        nc.tensor.matmul(
                            G[:, half * 1024 + q * 512:
                              half * 1024 + (q + 1) * 512],
                            XR[:, t * 128:(t + 1) * 128],
                            XQ[:, q * 512:(q + 1) * 512],
                            start=True, stop=True)
                es = []
                for h in range(NH):
                    E = ew.tile([128, 2048], bf16, tag="e")
                    nc.scalar.activation(
                        E[:], G[:], mybir.ActivationFunctionType.Exp,
                        scale=SCL[:, h:h + 1])
                    es.append(E)
                e_tiles[st] = es

            def emit_z(st):
                g, s = divmod(st, 2)
                if s == 0:
                    zfA = ps_z.tile([32, 512], f32, tag="zfA", name="zfA")
                    zfB = ps_z.tile([32, 512], f32, tag="zfB", name="zfB")
                    zf_tiles[g] = (zfA, zfB)
                zfA, zfB = zf_tiles[g]
                for h in range(NH):
                    E = e_tiles[st][h]
                    first = (s == 0 and h == 0)
                    last = (s == 1 and h == NH - 1)
                    for half in (0, 1):
                        t = 2 * st + half
                        bo_t = BO[:, (h * NT + t) * 32:(h * NT + t) * 32 + 32]
                        nc.tensor.matmul(
                            zfA[:, :], bo_t,
                            E[:, half * 1024:half * 1024 + 512],
                            start=(first and half == 0),
                            stop=(last and half == 1))
                        nc.tensor.matmul(
                            zfB[:, :], bo_t,
                            E[:, half * 1024 + 512:half * 1024 + 1024],
                            start=(first and half == 0),
                            stop=(last and half == 1))

            def emit_recip(g):
                zfA, zfB = zf_tiles[g]
                zi = zn.tile([32, 1024], bf16, tag="zi")
                with nc.allow_low_precision("bf16 softmax denominators"):
                    nc.vector.reciprocal(zi[:, 0:512], zfA[:])
                    nc.vector.reciprocal(zi[:, 512:1024], zfB[:])
                zi_tiles[g] = zi
                del zf_tiles[g]

            def emit_av(st):
                g, s = divmod(st, 2)
                zi = zi_tiles[g]
                for h in range(NH):
                    E = e_tiles[st][h]
                    zb = zbp.tile([128, 2048], bf16, tag="zb")
                    for half in (0, 1):
                        dt = 2 * s + half
                        r0 = h * 8 + 2 * dt
                        # zi row free layout: [jkA 0:512 | jkB 512:1024]
                        # = full jk 1024 of tile t=2st+half
                        src = zi[r0:r0 + 2, :].unsqueeze(1).broadcast_to(
                            (2, 64, 1024))
                        eng = nc.sync if (h + half) % 2 == 0 else nc.scalar
                        eng.dma_start(
                            zb[:, half * 1024:(half + 1) * 1024], src)
                    pt = ptp.tile([128, 2048], bf16, tag="pt")
                    nc.vector.tensor_mul(pt[:], E[:], zb[:])
                    first = (st == 0 and h == 0)
                    last = (st == NST - 1 and h == NH - 1)
                    for half in (0, 1):
                        t = 2 * st + half
                        vt_t = VT[:, (h * NT + t) * D:(h * NT + t + 1) * D]
                        for q in (0, 1):
                            nc.tensor.matmul(
                                av[:, q * 512:(q + 1) * 512],
                                vt_t,
                                pt[:, half * 1024 + q * 512:
                                   half * 1024 + (q + 1) * 512],
                                start=(first and half == 0),
                                stop=(last and half == 1))
                del e_tiles[st]

            for it in range(NST + 2):
                if it < NST:
                    emit_gram_exp(it)
                if 1 <= it <= NST:
                    emit_z(it - 1)
                    if (it - 1) % 2 == 1:
                        emit_recip((it - 1) // 2)
                if 2 <= it:
                    emit_av(it - 2)

            ob = obp.tile([D, JQ], f32, tag="ob")
            nc.vector.tensor_copy(ob[:], av[:])
            nc.sync.dma_start(out_d[:], ob[:])

    _split_excess_waits(nc)
    return nc


_NC = None


def _get_program():
    global _NC
    if _NC is None:
        _NC = _build_program()
    return _NC


def _make_in_maps(x, wq, bq, wk, bk, wv, bv):
    x = np.asarray(x, dtype=np.float32)
    x2 = x.reshape(B, D, HW)
    wq, bq, wk, bk, wv, bv = [
        np.asarray(a, dtype=np.float32) for a in (wq, bq, wk, bk, wv, bv)]

    per_batch = {}
    for b in range(B):
        X = x2[b]                              # [96, 4096]
        r = X.sum(axis=0)                      # [4096]
        vt = np.zeros((128, NH * NT * D), dtype=BF16)
        bo = np.zeros((128, NH * NT * 32), dtype=BF16)
        scl = np.zeros((128, NH), dtype=np.float32)
        for h in range(NH):
            alpha = wq[h] * wk[h]
            beta = bq[h] * wk[h]
            f = np.exp(SCALE * beta * r).astype(BF16)       # [4096]
            f32f = f.astype(np.float32)
            Vp = ((wv[h] * X + bv[h]) * f32f[None, :]).astype(BF16)
            vt[:, h * NT * D:(h + 1) * NT * D] = (
                Vp.reshape(D, NT, 128).transpose(2, 1, 0).reshape(128, NT * D))
            ft = f.reshape(NT, 128).T                        # [128, NT]
            for t in range(NT):
                dt = t % 4                   # tile index within group
                blk = bo[:, (h * NT + t) * 32:(h * NT + t) * 32 + 32]
                blk[0:64, h * 8 + 2 * dt] = ft[0:64, t]
                blk[64:128, h * 8 + 2 * dt + 1] = ft[64:128, t]
            scl[:, h] = SCALE * alpha
        per_batch[b] = (np.ascontiguousarray(X), vt, bo, scl)

    in_maps = []
    for c in range(NCORES):
        b, jq = divmod(c, NH)
        X, vt, bo, scl = per_batch[b]
        in_maps.append({
            "xr": X,
            "xq": np.ascontiguousarray(X[:, jq * JQ:(jq + 1) * JQ]),
            "vt": vt,
            "bo": bo,
            "scl": scl,
        })
    return in_maps


def kernel(x, wq, bq, wk, bk, wv, bv):
    nc = _get_program()
    in_maps = _make_in_maps(x, wq, bq, wk, bk, wv, bv)
    res = run_bass_kernel_spmd(nc, in_maps, core_ids=list(range(NCORES)))
    out = np.zeros((B, 1, D, 64, 64), dtype=np.float32)
    for c in range(NCORES):
        b, jq = divmod(c, NH)
        blk = res.results[c]["out"]            # [96, 1024]
        out[b, 0].reshape(D, HW)[:, jq * JQ:(jq + 1) * JQ] = blk
    return out
